# revision 1
# baseline (speedup 1.0000x reference)
"""Trainium2 Bass kernel for capsule agreement-routing (dynamic routing).

Reference computation (per batch b, per spatial location (o,h,w)):
    P = pred[b,o,h,w]            # [I, D]
    bb = b_in[0,o,h,w]           # [I]   (shared across batch)
    c = softmax(bb); out = squash(P^T c)
    repeat num_iterations times:
        bb += P @ out; c = softmax(bb); out = squash(P^T c)

Every location is independent, so the kernel is data-parallel over
batch (B=8 -> one batch element per NeuronCore) and tiles the 32768
locations per core as [128 partitions x F locations] SBUF tiles with
the full [I,D]=[16,16] block in the free dimension.  All routing
iterations run on a tile while it is resident, so pred streams from
HBM exactly once.

Softmax is computed without per-iteration max subtraction (the input
bias is max-subtracted once, which is exactly softmax-invariant; the
iteration increments are O(5), so exp stays in range) and the 1/Z
normalization is folded into the squash scale:
    out = q / ((Z^2+q) * sqrt(q + eps*Z^2)) * s_un
with s_un = sum_i e_i P_i, q = |s_un|^2, Z = sum_i e_i.
"""

import sys

for _p in ("/opt/trn_rl_repo",):
    if _p not in sys.path:
        sys.path.insert(0, _p)

from contextlib import ExitStack

import numpy as np

from concourse import bacc, mybir, tile
from concourse.bass_utils import run_bass_kernel_spmd

EPS = 1e-7
P = 128            # SBUF partitions
F = 16             # locations per partition per tile
I = 16             # input capsules per location
D = 16             # capsule dim
TILE_LOCS = P * F
N_CORES = 8

f32 = mybir.dt.float32
bf16 = mybir.dt.bfloat16


def _build_program(n_iter: int, locs: int):
    nc = bacc.Bacc("TRN2", target_bir_lowering=False, debug=False,
                   num_devices=N_CORES)
    pred_d = nc.dram_tensor("pred", [locs, I, D], f32, kind="ExternalInput").ap()
    b_d = nc.dram_tensor("b", [locs, I], f32, kind="ExternalInput").ap()
    out_d = nc.dram_tensor("out", [locs, I], f32, kind="ExternalOutput").ap()

    n_tiles = locs // TILE_LOCS
    assert n_tiles * TILE_LOCS == locs

    with tile.TileContext(nc) as tc, ExitStack() as ctx:
        pred_pool = ctx.enter_context(tc.tile_pool(name="pred", bufs=2))
        tmp_pool = ctx.enter_context(tc.tile_pool(name="tmp", bufs=2))
        tmp2_pool = ctx.enter_context(tc.tile_pool(name="tmp2", bufs=2))
        sm_pool = ctx.enter_context(tc.tile_pool(name="sm", bufs=2))

        for t in range(n_tiles):
            lo = t * TILE_LOCS
            pred_t = pred_pool.tile([P, F, I, D], f32, tag="pred")
            bt = sm_pool.tile([P, F, I], f32, tag="bt")
            nc.sync.dma_start(
                out=pred_t[:],
                in_=pred_d[lo:lo + TILE_LOCS].rearrange("(p f) i d -> p f i d", p=P),
            )
            nc.sync.dma_start(
                out=bt[:],
                in_=b_d[lo:lo + TILE_LOCS].rearrange("(p f) i -> p f i", p=P),
            )

            # one-time max subtraction (exactly softmax-invariant)
            m = sm_pool.tile([P, F], f32, tag="m")
            nc.vector.tensor_reduce(
                out=m[:], in_=bt[:], axis=mybir.AxisListType.X,
                op=mybir.AluOpType.max,
            )
            nc.vector.tensor_sub(bt[:], bt[:], m[:].unsqueeze(2).broadcast_to([P, F, I]))

            e = sm_pool.tile([P, F, I], f32, tag="e")
            z = sm_pool.tile([P, F], f32, tag="z")
            s_un = sm_pool.tile([P, F, D], f32, tag="s")
            out_t = sm_pool.tile([P, F, D], f32, tag="out")
            a = sm_pool.tile([P, F, I], f32, tag="a")

            def weighted_and_squash():
                nc.scalar.activation(e[:], bt[:], mybir.ActivationFunctionType.Exp)
                nc.vector.tensor_reduce(
                    out=z[:], in_=e[:], axis=mybir.AxisListType.X,
                    op=mybir.AluOpType.add,
                )
                # tmp2[p,f,d,i] = pred[p,f,i,d] * e[p,f,i]   (transposed write)
                tmp2 = tmp2_pool.tile([P, F, D, I], f32, tag="tmp2")
                nc.vector.tensor_mul(
                    tmp2[:].transpose([0, 1, 3, 2]),
                    pred_t[:],
                    e[:].unsqueeze(3).broadcast_to([P, F, I, D]),
                )
                nc.vector.tensor_reduce(
                    out=s_un[:], in_=tmp2[:], axis=mybir.AxisListType.X,
                    op=mybir.AluOpType.add,
                )
                # squash with folded 1/Z
                s2 = tmp2_pool.tile([P, F, D], f32, tag="s2")
                q = sm_pool.tile([P, F], f32, tag="q")
                nc.scalar.activation(s2[:], s_un[:],
                                     mybir.ActivationFunctionType.Square)
                nc.vector.tensor_reduce(
                    out=q[:], in_=s2[:], axis=mybir.AxisListType.X,
                    op=mybir.AluOpType.add,
                )
                z2 = sm_pool.tile([P, F], f32, tag="z2")
                t1 = sm_pool.tile([P, F], f32, tag="t1")
                t2 = sm_pool.tile([P, F], f32, tag="t2")
                alpha = sm_pool.tile([P, F], f32, tag="alpha")
                nc.vector.tensor_mul(z2[:], z[:], z[:])
                nc.vector.tensor_add(t1[:], z2[:], q[:])
                nc.vector.scalar_tensor_tensor(
                    out=t2[:], in0=z2[:], scalar=EPS, in1=q[:],
                    op0=mybir.AluOpType.mult, op1=mybir.AluOpType.add,
                )
                nc.scalar.activation(t2[:], t2[:], mybir.ActivationFunctionType.Sqrt)
                nc.vector.tensor_mul(t1[:], t1[:], t2[:])
                nc.vector.reciprocal(t1[:], t1[:])
                nc.vector.tensor_mul(alpha[:], q[:], t1[:])
                nc.vector.tensor_mul(
                    out_t[:], s_un[:], alpha[:].unsqueeze(2).broadcast_to([P, F, D])
                )

            weighted_and_squash()
            for _ in range(n_iter):
                tmp = tmp_pool.tile([P, F, I, D], f32, tag="tmp")
                nc.vector.tensor_mul(
                    tmp[:], pred_t[:],
                    out_t[:].unsqueeze(2).broadcast_to([P, F, I, D]),
                )
                nc.vector.tensor_reduce(
                    out=a[:], in_=tmp[:], axis=mybir.AxisListType.X,
                    op=mybir.AluOpType.add,
                )
                nc.vector.tensor_add(bt[:], bt[:], a[:])
                weighted_and_squash()

            nc.sync.dma_start(
                out=out_d[lo:lo + TILE_LOCS].rearrange("(p f) i -> p f i", p=P),
                in_=out_t[:],
            )

    nc.compile()
    return nc


_CACHE: dict = {}


def _get_program(n_iter: int, locs: int):
    key = (n_iter, locs)
    if key not in _CACHE:
        _CACHE[key] = _build_program(n_iter, locs)
    return _CACHE[key]


def _run(pred, b, n_iter, trace=False):
    pred = np.ascontiguousarray(np.asarray(pred, dtype=np.float32))
    b = np.ascontiguousarray(np.asarray(b, dtype=np.float32))
    B, O, H, W, I_, D_ = pred.shape
    assert (I_, D_) == (I, D) and B == N_CORES and b.shape == (1, O, H, W, I)
    locs = O * H * W
    nc = _get_program(int(n_iter), locs)

    b_flat = b.reshape(locs, I)
    in_maps = [
        {"pred": pred[k].reshape(locs, I, D), "b": b_flat}
        for k in range(N_CORES)
    ]
    res = run_bass_kernel_spmd(nc, in_maps, list(range(N_CORES)), trace=trace)
    out = np.stack([res.results[k]["out"].reshape(O, H, W, I) for k in range(N_CORES)])
    return out, res


def kernel(**inputs) -> np.ndarray:
    pred = inputs["tensor_of_prediction_vector"]
    b = inputs["b"]
    n_iter = int(np.asarray(inputs["num_iterations"]))
    out, _ = _run(pred, b, n_iter, trace=False)
    return out


def kernel_traced(**inputs):
    """Like kernel() but also returns the BassKernelResults (exec_time_ns)."""
    pred = inputs["tensor_of_prediction_vector"]
    b = inputs["b"]
    n_iter = int(np.asarray(inputs["num_iterations"]))
    return _run(pred, b, n_iter, trace=True)


# revision 2
# speedup vs baseline: 1.2326x; 1.2326x over previous
"""Trainium2 Bass kernel for capsule agreement-routing — optimized builder.

Per 128xF-location tile (all routing iterations fused while resident):
  - pred streams HBM->SBUF once (f32), cast once to fp16 in both the
    natural (f,i,d) and transposed (f,d,i) orders on ScalarE (sub-word
    strided access on DVE is ~2x; ScalarE is otherwise idle).
  - agreement product tmp = pred (x) out, (f,i,d)-natural, fp16, 2x mode.
  - the agreement increment is max-subtracted (exactly softmax-invariant)
    so fp16 weights never overflow; on even iterations of the b==0 path a
    constant shift folded into the exp bias replaces the max-reduction.
  - weighted products keep a multiplicative recurrence
        tmp2_{t+1}[l,i,d] = tmp2_t[l,i,d] * g_t[l,i],  g = exp(a - shift)
    stored (f,d,i)-transposed so the recurrence multiply and the
    i-reduction tree are both 2x-eligible.
  - reductions are pairwise trees in fp16 (final level -> f32).
  - softmax 1/Z is folded into the squash scale (scale-invariant):
        out = q/((Z^2+q)*sqrt(q+eps*Z^2)) * s_un.
  - b == 0 specialization (runtime-guarded): initial weights are uniform,
    so tmp2_0 is pred itself (cast writes straight into the tmp2 slot)
    and the init squash uses Z0 = I as a constant.
  - tiles are emitted interleaved in groups so one tile's DVE work hides
    another's ScalarE-dependency stalls (scheduler priority follows
    emission order).
"""

import sys

for _p in ("/opt/trn_rl_repo",):
    if _p not in sys.path:
        sys.path.insert(0, _p)

from contextlib import ExitStack

import numpy as np

from concourse import bacc, mybir, tile
from concourse.bass_utils import run_bass_kernel_spmd

EPS = 1e-7
P = 128
I = 16
D = 16
N_CORES = 8

f32 = mybir.dt.float32
AX = mybir.AxisListType.X
ADD = mybir.AluOpType.add
MULT = mybir.AluOpType.mult
MAX = mybir.AluOpType.max
ACTF = mybir.ActivationFunctionType

CFG = dict(
    F=16,                # locations per partition per tile
    dt16="float16",      # 16-bit compute dtype
    mult1_16=True,       # agreement product in 16-bit
    tree1_16=True,       # agreement reduction as 16-bit tree (else f32 TR)
    tree2_16=True,       # weighted reduction as 16-bit tree (else f32 TR)
    shift=True,          # max-subtraction of the increment
    interleave=3,        # tiles emitted round-robin in groups of this size
    const_shift=5.5,     # b==0 path: even iters use this constant shift
)


def _tree_reduce(nc, pool, src, F_, n_seg, seg, out_f32, tag, dt16):
    """Pairwise-reduce last axis: src [P,F_,n_seg,seg] -> out_f32 [P,F_,n_seg]."""
    if dt16 is None:
        nc.vector.tensor_reduce(out=out_f32[:], in_=src[:], axis=AX, op=ADD)
        return
    cur = src
    w = seg
    while w > 2:
        w //= 2
        nxt = pool.tile([P, F_, n_seg, w], dt16, tag=f"{tag}{w}")
        nc.vector.tensor_add(nxt[:], cur[:, :, :, 0:w], cur[:, :, :, w : 2 * w])
        cur = nxt
    nc.vector.tensor_add(
        out_f32[:], cur[:, :, :, 0:1].squeeze(3), cur[:, :, :, 1:2].squeeze(3)
    )


def build_tc(tc, pred_d, b_d, out_d, n_iter, locs, cfg=CFG, zb=False):
    nc = tc.nc
    F = cfg["F"]
    dt16 = getattr(mybir.dt, cfg["dt16"])
    CSH = float(cfg.get("const_shift", 0.0))
    TILE_LOCS = P * F
    n_tiles = locs // TILE_LOCS
    assert n_tiles * TILE_LOCS == locs

    with ExitStack() as ctx:
        predf_pool = ctx.enter_context(tc.tile_pool(name="predf", bufs=2))
        predh_pool = ctx.enter_context(tc.tile_pool(name="predh", bufs=3))
        tmp_pool = ctx.enter_context(tc.tile_pool(name="tmp", bufs=2))
        tmp2_pool = ctx.enter_context(tc.tile_pool(name="tmp2", bufs=3))
        tree_pool = ctx.enter_context(tc.tile_pool(name="tree", bufs=2))
        sm_pool = ctx.enter_context(tc.tile_pool(name="sm", bufs=3))

        const_pool = ctx.enter_context(tc.tile_pool(name="const", bufs=1))
        csh_bias = None
        if CSH > 0.0:
            csh_bias = const_pool.tile([P, 1], f32, tag="csh")
            nc.vector.memset(csh_bias[:], -CSH)

        def tile_prog(t):
            lo = t * TILE_LOCS
            pred_src = pred_d[lo : lo + TILE_LOCS].rearrange(
                "(p f) i d -> p f i d", p=P
            )
            pred_f = predf_pool.tile([P, F, I, D], f32, tag="predf")
            nc.sync.dma_start(out=pred_f[:], in_=pred_src)
            pred_h = predh_pool.tile([P, F, I, D], dt16, tag="predh")
            tmp2 = tmp2_pool.tile([P, F, D, I], dt16, tag="tmp2")
            e = sm_pool.tile([P, F, I], dt16, tag="e")
            z = sm_pool.tile([P, F], f32, tag="z")
            s_un = sm_pool.tile([P, F, D], f32, tag="s")
            out_t = sm_pool.tile([P, F, D], f32, tag="out")
            out_h = sm_pool.tile([P, F, D], dt16, tag="outh")
            a = sm_pool.tile([P, F, I], f32, tag="a")

            if zb:
                # b == 0: uniform initial weights (e0=1, Z0=I) -> tmp2_0 is
                # pred itself; cast straight into the tmp2 slot.
                nc.scalar.activation(pred_h[:], pred_f[:], ACTF.Copy)
                nc.scalar.activation(
                    tmp2[:], pred_f[:].transpose([0, 1, 3, 2]), ACTF.Copy
                )
                yield
            else:
                pred_ht = predh_pool.tile([P, F, D, I], dt16, tag="predht")
                nc.scalar.activation(pred_h[:], pred_f[:], ACTF.Copy)
                nc.scalar.activation(
                    pred_ht[:], pred_f[:].transpose([0, 1, 3, 2]), ACTF.Copy
                )
                yield
                bt = sm_pool.tile([P, F, I], f32, tag="bt")
                nc.sync.dma_start(
                    out=bt[:],
                    in_=b_d[lo : lo + TILE_LOCS].rearrange(
                        "(p f) i -> p f i", p=P
                    ),
                )
                m = sm_pool.tile([P, F], f32, tag="m")
                nc.vector.tensor_reduce(out=m[:], in_=bt[:], axis=AX, op=MAX)
                nc.vector.tensor_sub(
                    bt[:], bt[:], m[:].unsqueeze(2).broadcast_to([P, F, I])
                )
                yield
                nc.scalar.activation(e[:], bt[:], ACTF.Exp)
                nc.vector.tensor_mul(
                    tmp2[:],
                    pred_ht[:],
                    e[:].unsqueeze(2).broadcast_to([P, F, D, I]),
                )
                yield

            def z_s_and_squash(last, init_const_z):
                if not init_const_z:
                    nc.vector.tensor_reduce(out=z[:], in_=e[:], axis=AX, op=ADD)
                yield from _gen_noop()
                _tree_reduce(
                    nc, tree_pool, tmp2, F, D, I, s_un, "t2_",
                    dt16 if cfg["tree2_16"] else None,
                )
                yield
                s2 = tree_pool.tile([P, F, D], f32, tag="s2")
                q = sm_pool.tile([P, F], f32, tag="q")
                nc.scalar.activation(s2[:], s_un[:], ACTF.Square)
                nc.vector.tensor_reduce(out=q[:], in_=s2[:], axis=AX, op=ADD)
                t1 = sm_pool.tile([P, F], f32, tag="t1")
                t2 = sm_pool.tile([P, F], f32, tag="t2s")
                alpha = sm_pool.tile([P, F], f32, tag="alpha")
                if init_const_z:
                    z0sq = float(I * I)
                    nc.vector.tensor_scalar_add(t1[:], q[:], z0sq)
                    nc.vector.tensor_scalar_add(t2[:], q[:], EPS * z0sq)
                else:
                    z2 = sm_pool.tile([P, F], f32, tag="z2")
                    nc.vector.tensor_mul(z2[:], z[:], z[:])
                    nc.vector.tensor_add(t1[:], z2[:], q[:])
                    nc.vector.scalar_tensor_tensor(
                        out=t2[:], in0=z2[:], scalar=EPS, in1=q[:],
                        op0=MULT, op1=ADD,
                    )
                nc.scalar.activation(t2[:], t2[:], ACTF.Sqrt)
                yield
                nc.vector.tensor_mul(t1[:], t1[:], t2[:])
                nc.vector.reciprocal(t1[:], t1[:])
                nc.vector.tensor_mul(alpha[:], q[:], t1[:])
                nc.vector.tensor_mul(
                    (out_t if last else out_h)[:],
                    s_un[:],
                    alpha[:].unsqueeze(2).broadcast_to([P, F, D]),
                )
                yield

            def _gen_noop():
                return
                yield

            yield from z_s_and_squash(last=False, init_const_z=zb)

            for it in range(n_iter):
                if cfg["mult1_16"]:
                    tmp = tmp_pool.tile([P, F, I, D], dt16, tag="tmp")
                    nc.vector.tensor_mul(
                        tmp[:], pred_h[:],
                        out_h[:].unsqueeze(2).broadcast_to([P, F, I, D]),
                    )
                else:
                    tmp = tmp_pool.tile([P, F, I, D], f32, tag="tmpf")
                    nc.vector.tensor_mul(
                        tmp[:], pred_f[:],
                        out_t[:].unsqueeze(2).broadcast_to([P, F, I, D]),
                    )
                yield
                _tree_reduce(
                    nc, tree_pool, tmp, F, I, D, a, "t1_",
                    dt16 if cfg["tree1_16"] else None,
                )
                yield
                # fp16 range control: exact max-shift (softmax-invariant);
                # on even iterations of the b==0 path a constant shift
                # folded into the exp bias is sufficient and free.
                const_sh = zb and CSH > 0.0 and (it % 2 == 0)
                first_e = zb and it == 0
                g = e if first_e else sm_pool.tile([P, F, I], dt16, tag="g")
                if cfg["shift"] and not const_sh:
                    k = sm_pool.tile([P, F], f32, tag="k")
                    nc.vector.tensor_reduce(out=k[:], in_=a[:], axis=AX, op=MAX)
                    nc.vector.tensor_sub(
                        a[:], a[:], k[:].unsqueeze(2).broadcast_to([P, F, I])
                    )
                if const_sh:
                    nc.scalar.activation(g[:], a[:], ACTF.Exp, bias=csh_bias[:])
                else:
                    nc.scalar.activation(g[:], a[:], ACTF.Exp)
                yield
                if not first_e:
                    nc.vector.tensor_mul(e[:], e[:], g[:])
                nc.vector.tensor_mul(
                    tmp2[:], tmp2[:],
                    g[:].unsqueeze(2).broadcast_to([P, F, D, I]),
                )
                yield
                yield from z_s_and_squash(
                    last=(it == n_iter - 1), init_const_z=False
                )

            nc.sync.dma_start(
                out=out_d[lo : lo + TILE_LOCS].rearrange("(p f) i -> p f i", p=P),
                in_=out_t[:],
            )

        W = max(1, int(cfg.get("interleave", 1)))
        for base in range(0, n_tiles, W):
            progs = [tile_prog(t) for t in range(base, min(base + W, n_tiles))]
            while progs:
                done = []
                for g in progs:
                    try:
                        next(g)
                    except StopIteration:
                        done.append(g)
                for g in done:
                    progs.remove(g)


def _build_program(n_iter: int, locs: int, zb: bool, cfg=CFG):
    nc = bacc.Bacc(
        "TRN2", target_bir_lowering=False, debug=False, num_devices=N_CORES
    )
    pred_d = nc.dram_tensor("pred", [locs, I, D], f32, kind="ExternalInput").ap()
    b_d = nc.dram_tensor("b", [locs, I], f32, kind="ExternalInput").ap()
    out_d = nc.dram_tensor("out", [locs, I], f32, kind="ExternalOutput").ap()
    with tile.TileContext(nc) as tc:
        build_tc(tc, pred_d, b_d, out_d, n_iter, locs, cfg, zb=zb)
    nc.compile()
    return nc


_CACHE: dict = {}


def _get_program(n_iter: int, locs: int, zb: bool):
    key = (n_iter, locs, zb)
    if key not in _CACHE:
        _CACHE[key] = _build_program(n_iter, locs, zb)
    return _CACHE[key]


def _run(pred, b, n_iter, trace=False):
    pred = np.ascontiguousarray(np.asarray(pred, dtype=np.float32))
    b = np.ascontiguousarray(np.asarray(b, dtype=np.float32))
    B, O, H, W_, I_, D_ = pred.shape
    assert (I_, D_) == (I, D) and B == N_CORES and b.shape == (1, O, H, W_, I)
    locs = O * H * W_
    zb = not np.any(b)
    nc = _get_program(int(n_iter), locs, zb)
    b_flat = b.reshape(locs, I)
    in_maps = [
        {"pred": pred[k].reshape(locs, I, D), "b": b_flat} for k in range(N_CORES)
    ]
    res = run_bass_kernel_spmd(nc, in_maps, list(range(N_CORES)), trace=trace)
    out = np.stack(
        [res.results[k]["out"].reshape(O, H, W_, I) for k in range(N_CORES)]
    )
    return out, res


def kernel(**inputs) -> np.ndarray:
    pred = inputs["tensor_of_prediction_vector"]
    b = inputs["b"]
    n_iter = int(np.asarray(inputs["num_iterations"]))
    out, _ = _run(pred, b, n_iter, trace=False)
    return out


def kernel_traced(**inputs):
    pred = inputs["tensor_of_prediction_vector"]
    b = inputs["b"]
    n_iter = int(np.asarray(inputs["num_iterations"]))
    return _run(pred, b, n_iter, trace=True)
